# revision 11
# baseline (speedup 1.0000x reference)
"""Multi-head attention Bass kernel for Trainium2, 8-core SPMD.

Problem: B=4, S=2048, D=1024, H=16 heads (dh=64), boolean attention mask.
  out = softmax(mask ? -1e9 : (Q Kt / 8)) V -> @ Wo + bo

Sharding: 8 cores = 4 batches x 2 head-groups. Core c handles batch c//2,
heads [8*(c%2), 8*(c%2)+8). Host transposes activations to [D, S] so the
PE (which contracts along partitions) consumes them directly; the keep-mask
(~mask) is host-transposed to [head, kchunk, k, q] uint8. Per-core partial
outputs Y_c = (attn_heads @ Wo_slice) are summed pairwise on the host
(tensor-parallel "all-reduce"), and bias terms (bv@Wo + bo) are added once.

Device pipeline per core (all matmuls in fp32r: full PE rate, fp32 storage):
  A) QT/KT = Wq/Wk slices applied to queryT/keyT (+bias via DVE);
     V[t,d] = valueT chunks (stationary) x Wv (moving), stored with a
     ones-column per head (65-stride) so PV emits row-sums for free.
  B) per (head, q-block, k-chunk): S^T = KT' QT -> PSUM; ACT exp(s/8);
     DVE multiply by keep-mask (u8); PV accumulates O^T[65, q] where
     row 64 = sum_k exp. Evict via SBUF stage + DMA (partition relayout).
  C) recip = 1/sums; broadcast per-head recip rows onto 128 partitions with
     a 0/1 pattern matmul; normalize O^T in place; Y = O^T chunks @ Wo.
"""

import sys

if "/opt/trn_rl_repo" not in sys.path:
    sys.path.insert(0, "/opt/trn_rl_repo")

import numpy as np

B, S, D, H = 4, 2048, 1024, 16
DH = D // H            # 64 per-head dim
HC = H // 2            # 8 heads per core
DHC = HC * DH          # 512 per-core head dims
NCORES = 8
KC = S // 128          # 16 k chunks
OC = DHC // 128        # 4 output chunks for Q/K projections
TC = S // 128          # 16 t chunks
TB = S // 512          # 4 t blocks for projection staging
QB = S // 1024         # 2 q blocks in attention

_CACHE = {}


def _emit(nc, tc, tile, mybir):
    F32 = mybir.dt.float32
    U8 = mybir.dt.uint8
    F32R = mybir.dt.float32r
    mult = mybir.AluOpType.mult
    Exp = mybir.ActivationFunctionType.Exp

    BF = mybir.dt.bfloat16
    MF = BF          # attention-side matmul operand dtype
    OF = F32R        # output-path matmul operand dtype

    def r(ap):
        return ap

    qT = nc.dram_tensor("qT", [D, S], MF, kind="ExternalInput")
    kT = nc.dram_tensor("kT", [D, S], MF, kind="ExternalInput")
    vT = nc.dram_tensor("vT", [D, S], MF, kind="ExternalInput")
    mk = nc.dram_tensor("mk", [HC, KC, 128, S], U8, kind="ExternalInput")
    wq = nc.dram_tensor("wq", [D, DHC], MF, kind="ExternalInput")
    wk = nc.dram_tensor("wk", [D, DHC], MF, kind="ExternalInput")
    wv = nc.dram_tensor("wv", [D, DHC], MF, kind="ExternalInput")
    wo = nc.dram_tensor("wo", [DHC, D], OF, kind="ExternalInput")
    bq = nc.dram_tensor("bq", [DHC], F32, kind="ExternalInput")
    bk = nc.dram_tensor("bk", [DHC], F32, kind="ExternalInput")
    patd = nc.dram_tensor("patd", [HC, DHC], OF, kind="ExternalInput")
    y = nc.dram_tensor("y", [S, D], F32, kind="ExternalOutput")

    with tc.tile_pool(name="longp", bufs=1) as lp:
        sums_sb = lp.tile([HC, S], OF, name="sums", tag="sums")
        otn_sb = [lp.tile([128, S], OF, name=f"otn{i}", tag=f"otn{i}") for i in range(OC)]
        qkv_scope = tc.tile_pool(name="qkvp", bufs=1)
        pp = qkv_scope.__enter__()
        qt_sb = [pp.tile([128, S], MF, name=f"qt{i}", tag=f"qt{i}") for i in range(OC)]
        kt_sb = [pp.tile([128, S], MF, name=f"kt{i}", tag=f"kt{i}") for i in range(OC)]
        va_sb = [pp.tile([128, HC * 65], MF, name=f"va{i}", tag=f"va{i}") for i in range(TC)]

        # ---------------- Phase A: projections ----------------
        with (
            tc.tile_pool(name="wpool", bufs=10) as wp,
            tc.tile_pool(name="stage", bufs=12) as sp_,
            tc.tile_pool(name="biasp", bufs=1) as bp,
            tc.tile_pool(name="apsum", bufs=4, space="PSUM") as ap_,
        ):
            bq_sb = bp.tile([128, OC], F32, name="bq", tag="bq")
            bk_sb = bp.tile([128, OC], F32, name="bk", tag="bk")
            nc.sync.dma_start(bq_sb[:], bq.rearrange("(c p) -> p c", p=128))
            nc.sync.dma_start(bk_sb[:], bk.rearrange("(c p) -> p c", p=128))
            for i in range(TC):
                nc.gpsimd.memset(
                    va_sb[i].rearrange("p (h w) -> p h w", w=65)[:, :, 64:65]
                    .bitcast(mybir.dt.uint16),
                    0x3F80,
                )

            for name, srcd, wd, dst, bias in (
                ("wq", qT, wq, qt_sb, bq_sb),
                ("wk", kT, wk, kt_sb, bk_sb),
                ("wv", vT, wv, None, None),
            ):
                wch = []
                for i in range(8):
                    w = wp.tile([128, DHC], MF, name="w", tag="w")
                    nc.sync.dma_start(w[:], wd[i * 128:(i + 1) * 128, :])
                    wch.append(w)
                for tb in range(TB):
                    t0 = tb * 512
                    stg = []
                    for Dc in range(8):
                        st = sp_.tile([128, 512], MF, name="stage", tag="stage")
                        nc.sync.dma_start(
                            st[:], srcd[Dc * 128:(Dc + 1) * 128, t0:t0 + 512]
                        )
                        stg.append(st)
                    if name != "wv":
                        for oc in range(OC):
                            ps = ap_.tile([128, 512], F32, name="aps", tag="aps")
                            for Dc in range(8):
                                nc.tensor.matmul(
                                    ps[:],
                                    r(wch[Dc][:, oc * 128:(oc + 1) * 128]),
                                    r(stg[Dc][:]),
                                    start=(Dc == 0),
                                    stop=(Dc == 7),
                                )
                            nc.vector.tensor_scalar_add(
                                dst[oc][:, t0:t0 + 512], ps[:], bias[:, oc:oc + 1]
                            )
                    else:
                        for ti in range(4):
                            tcn = tb * 4 + ti
                            ps = ap_.tile([128, DHC], F32, name="aps", tag="aps")
                            for Dc in range(8):
                                nc.tensor.matmul(
                                    ps[:],
                                    r(stg[Dc][:, ti * 128:(ti + 1) * 128]),
                                    r(wch[Dc][:]),
                                    start=(Dc == 0),
                                    stop=(Dc == 7),
                                )
                            nc.scalar.copy(
                                va_sb[tcn].rearrange("p (h w) -> p h w", w=65)[:, :, 0:64],
                                ps.rearrange("p (h w) -> p h w", w=64)[:, :, :],
                            )

        # ---------------- Phase B: attention ----------------
        with (
            tc.tile_pool(name="maskp", bufs=3) as mp_,
            tc.tile_pool(name="maskbp", bufs=3) as mbp,
            tc.tile_pool(name="ptp", bufs=3) as ptp,
            tc.tile_pool(name="ostage", bufs=2) as osp,
            tc.tile_pool(name="spsum", bufs=2, space="PSUM") as sp2,
            tc.tile_pool(name="opsum", bufs=1, space="PSUM") as op2,
        ):
            for h in range(HC):
                hc, hp = h // 2, (h % 2) * 64
                ots = [op2.tile([65, 1024], F32, name=f"ot{qb}", tag=f"ot{qb}")
                       for qb in range(QB)]
                for kc in range(KC):
                    mt = mp_.tile([128, S], U8, name="mask", tag="mask")
                    nc.sync.dma_start(mt[:], mk[h, kc])
                    mb = mbp.tile([128, S], BF, name="maskb", tag="maskb")
                    nc.gpsimd.tensor_copy(mb[:], mt[:])
                    for qb in range(QB):
                        q0 = qb * 1024
                        st = sp2.tile([128, 1024], F32, name="st", tag="st")
                        for j in range(2):
                            nc.tensor.matmul(
                                st[:, j * 512:(j + 1) * 512],
                                r(kt_sb[hc][hp:hp + 64, kc * 128:(kc + 1) * 128]),
                                r(qt_sb[hc][hp:hp + 64, q0 + j * 512:q0 + (j + 1) * 512]),
                                start=True,
                                stop=True,
                            )
                        pt = ptp.tile([128, 1024], MF, name="pt", tag="pt")
                        nc.scalar.activation(pt[:], st[:], Exp, scale=0.125)
                        nc.vector.tensor_tensor(
                            pt[:], pt[:], mb[:, q0:q0 + 1024], op=mult
                        )
                        for j in range(2):
                            nc.tensor.matmul(
                                ots[qb][:, j * 512:(j + 1) * 512],
                                r(va_sb[kc][:, h * 65:h * 65 + 65]),
                                r(pt[:, j * 512:(j + 1) * 512]),
                                start=(kc == 0),
                                stop=(kc == KC - 1),
                                skip_group_check=True,
                            )
                for qb in range(QB):
                    q0 = qb * 1024
                    stg = osp.tile([65, 1024], OF, name="ostage", tag="ostage")
                    nc.scalar.copy(stg[:], ots[qb][:])
                    nc.sync.dma_start(
                        otn_sb[hc][hp:hp + 64, q0:q0 + 1024], stg[0:64, :]
                    )
                    nc.sync.dma_start(
                        sums_sb[h:h + 1, q0:q0 + 1024], stg[64:65, :]
                    )

        qkv_scope.__exit__(None, None, None)

        # ---------------- Phase C: normalize + output projection ----------------
        with (
            tc.tile_pool(name="rcp", bufs=1) as rcp,
            tc.tile_pool(name="wop", bufs=1) as wop,
            tc.tile_pool(name="ystage", bufs=3) as ysp,
            tc.tile_pool(name="cpsum", bufs=4, space="PSUM") as cp_,
        ):
            recip_sb = rcp.tile([HC, S], OF, name="recip", tag="recip")
            with nc.allow_low_precision(reason="fp32r holds full fp32 reciprocal"):
                nc.vector.reciprocal(recip_sb[:], sums_sb[:])
            pat_sb = rcp.tile([HC, DHC], OF, name="pat", tag="pat")
            nc.sync.dma_start(pat_sb[:], patd[:])
            pats = [pat_sb[:, dc * 128:(dc + 1) * 128] for dc in range(OC)]
            for dc in range(OC):
                for qs in range(4):
                    rps = cp_.tile([128, 512], F32, name="cps", tag="cps")
                    nc.tensor.matmul(
                        rps[:], r(pats[dc][:]), r(recip_sb[:, qs * 512:(qs + 1) * 512]),
                        start=True, stop=True,
                    )
                    nc.vector.tensor_tensor(
                        otn_sb[dc][:, qs * 512:(qs + 1) * 512],
                        otn_sb[dc][:, qs * 512:(qs + 1) * 512],
                        rps[:],
                        op=mult,
                    )
            wo_sb = [wop.tile([128, D], OF, name=f"wo{i}", tag=f"wo{i}") for i in range(OC)]
            for i in range(OC):
                nc.sync.dma_start(wo_sb[i][:], wo[i * 128:(i + 1) * 128, :])
            for t in range(TC):
                ys = ysp.tile([128, D], F32, name="ys", tag="ys")
                for half in range(2):
                    yps = cp_.tile([128, 512], F32, name="cps", tag="cps")
                    for dc in range(OC):
                        nc.tensor.matmul(
                            yps[:],
                            r(otn_sb[dc][:, t * 128:(t + 1) * 128]),
                            r(wo_sb[dc][:, half * 512:(half + 1) * 512]),
                            start=(dc == 0),
                            stop=(dc == OC - 1),
                        )
                    nc.scalar.copy(ys[:, half * 512:(half + 1) * 512], yps[:])
                nc.sync.dma_start(y[t * 128:(t + 1) * 128, :], ys[:])


def _build():
    if "nc" in _CACHE:
        return _CACHE["nc"]
    import concourse.mybir as mybir
    import concourse.tile as tile
    from concourse import bacc

    nc = bacc.Bacc(None, target_bir_lowering=False)
    with tile.TileContext(nc) as tc:
        _emit(nc, tc, tile, mybir)
    nc.compile()
    _CACHE["nc"] = nc
    return nc


def _pattern():
    """[h, d] -> 1.0 where global row dc*128+(d%128) belongs to head h."""
    p = np.zeros((HC, DHC), dtype=np.float32)
    for d in range(DHC):
        p[d // DH, d] = 1.0
    return p


def _prep_inputs(query, key, value, mask, Wq, Wk, Wv, Wo, bq, bk):
    """Build the 8 per-core input maps (host-side sharding/relayout)."""
    import ml_dtypes
    bf16 = ml_dtypes.bfloat16
    in_maps = []
    qT = [np.ascontiguousarray(query[b].T).astype(bf16) for b in range(B)]
    kTt = [np.ascontiguousarray(key[b].T).astype(bf16) for b in range(B)]
    vT = [np.ascontiguousarray(value[b].T).astype(bf16) for b in range(B)]
    # keep-mask, transposed to [b][h, k, q] then chunked [h, kc, 128, q]
    keep = (~mask).astype(np.uint8).reshape(B, S, H, S)
    for c in range(NCORES):
        b, g = c // 2, c % 2
        hs = slice(g * HC * DH, (g + 1) * HC * DH)
        mkc = np.ascontiguousarray(
            keep[b, :, g * HC:(g + 1) * HC, :].transpose(1, 2, 0)
        ).reshape(HC, KC, 128, S)
        in_maps.append({
            "qT": qT[b], "kT": kTt[b], "vT": vT[b], "mk": mkc,
            "wq": np.ascontiguousarray(Wq[:, hs]).astype(bf16),
            "wk": np.ascontiguousarray(Wk[:, hs]).astype(bf16),
            "wv": np.ascontiguousarray(Wv[:, hs]).astype(bf16),
            "wo": np.ascontiguousarray(Wo[hs, :]),
            "bq": np.ascontiguousarray(bq[hs]),
            "bk": np.ascontiguousarray(bk[hs]),
            "patd": _pattern(),
        })
    return in_maps


def kernel(query, key, value, mask, Wq, bq, Wk, bk, Wv, bv, Wo, bo, _trace=False):
    from concourse.bass_utils import run_bass_kernel_spmd

    query = np.asarray(query, dtype=np.float32)
    key = np.asarray(key, dtype=np.float32)
    value = np.asarray(value, dtype=np.float32)
    mask = np.asarray(mask)
    Wq = np.asarray(Wq, dtype=np.float32)
    Wk = np.asarray(Wk, dtype=np.float32)
    Wv = np.asarray(Wv, dtype=np.float32)
    Wo = np.asarray(Wo, dtype=np.float32)
    bq = np.asarray(bq, dtype=np.float32)
    bk = np.asarray(bk, dtype=np.float32)
    bv = np.asarray(bv, dtype=np.float32)
    bo = np.asarray(bo, dtype=np.float32)

    nc = _build()
    in_maps = _prep_inputs(query, key, value, mask, Wq, Wk, Wv, Wo, bq, bk)
    kwargs = {}
    if _trace:
        kwargs = {"trace": True}
    res = run_bass_kernel_spmd(nc, in_maps, core_ids=list(range(NCORES)), **kwargs)
    # attention weights sum to 1, so the V bias contributes bv @ Wo verbatim
    const_vec = (bv @ Wo + bo).astype(np.float32)
    out = np.empty((B, S, D), dtype=np.float32)
    for b in range(B):
        out[b] = res.results[2 * b]["y"] + res.results[2 * b + 1]["y"] + const_vec
    if _trace:
        return out, res
    return out


# revision 12
# speedup vs baseline: 1.7324x; 1.7324x over previous
"""Multi-head attention Bass kernel for Trainium2, 8-core SPMD.

Problem: B=4, S=2048, D=1024, H=16 heads (dh=64), boolean attention mask.
  out = softmax(mask ? -1e9 : (Q Kt / 8)) V -> @ Wo + bo

Sharding: 8 cores = 4 batches x 2 head-groups. Core c handles batch c//2,
heads [8*(c%2), 8*(c%2)+8). Host transposes activations to [D, S] so the
PE (which contracts along partitions) consumes them directly; the keep-mask
(~mask) is host-transposed to [head, kchunk, k, q] uint8. Per-core partial
outputs Y_c = (attn_heads @ Wo_slice) are summed pairwise on the host
(tensor-parallel "all-reduce"), and bias terms (bv@Wo + bo) are added once.

Device pipeline per core (all matmuls in fp32r: full PE rate, fp32 storage):
  A) QT/KT = Wq/Wk slices applied to queryT/keyT (+bias via DVE);
     V[t,d] = valueT chunks (stationary) x Wv (moving), stored with a
     ones-column per head (65-stride) so PV emits row-sums for free.
  B) per (head, q-block, k-chunk): S^T = KT' QT -> PSUM; ACT exp(s/8);
     DVE multiply by keep-mask (u8); PV accumulates O^T[65, q] where
     row 64 = sum_k exp. Evict via SBUF stage + DMA (partition relayout).
  C) recip = 1/sums; broadcast per-head recip rows onto 128 partitions with
     a 0/1 pattern matmul; normalize O^T in place; Y = O^T chunks @ Wo.
"""

import sys

if "/opt/trn_rl_repo" not in sys.path:
    sys.path.insert(0, "/opt/trn_rl_repo")

import numpy as np

B, S, D, H = 4, 2048, 1024, 16
DH = D // H            # 64 per-head dim
HC = H // 2            # 8 heads per core
DHC = HC * DH          # 512 per-core head dims
NCORES = 8
KC = S // 128          # 16 k chunks
OC = DHC // 128        # 4 output chunks for Q/K projections
TC = S // 128          # 16 t chunks
TB = S // 512          # 4 t blocks for projection staging
QB = S // 1024         # 2 q blocks in attention

_CACHE = {}


def _emit(nc, tc, tile, mybir):
    F32 = mybir.dt.float32
    U8 = mybir.dt.uint8
    F32R = mybir.dt.float32r
    mult = mybir.AluOpType.mult
    Exp = mybir.ActivationFunctionType.Exp

    BF = mybir.dt.bfloat16
    MF = BF          # attention-side matmul operand dtype
    OF = F32R        # output-path matmul operand dtype

    def r(ap):
        return ap

    qT = nc.dram_tensor("qT", [D, S], MF, kind="ExternalInput")
    kT = nc.dram_tensor("kT", [D, S], MF, kind="ExternalInput")
    vT = nc.dram_tensor("vT", [D, S], MF, kind="ExternalInput")
    mk = nc.dram_tensor("mk", [HC, KC, 128, S], U8, kind="ExternalInput")
    wq = nc.dram_tensor("wq", [D, DHC], MF, kind="ExternalInput")
    wk = nc.dram_tensor("wk", [D, DHC], MF, kind="ExternalInput")
    wv = nc.dram_tensor("wv", [D, DHC], MF, kind="ExternalInput")
    wo = nc.dram_tensor("wo", [DHC, D], OF, kind="ExternalInput")
    bq = nc.dram_tensor("bq", [DHC], F32, kind="ExternalInput")
    bk = nc.dram_tensor("bk", [DHC], F32, kind="ExternalInput")
    patd = nc.dram_tensor("patd", [HC, DHC], OF, kind="ExternalInput")
    y = nc.dram_tensor("y", [S, D], F32, kind="ExternalOutput")

    with tc.tile_pool(name="longp", bufs=1) as lp:
        sums_sb = lp.tile([HC, S], OF, name="sums", tag="sums")
        otn_sb = [lp.tile([128, S], OF, name=f"otn{i}", tag=f"otn{i}") for i in range(OC)]
        qkv_scope = tc.tile_pool(name="qkvp", bufs=1)
        pp = qkv_scope.__enter__()
        qt_sb = [pp.tile([128, S], MF, name=f"qt{i}", tag=f"qt{i}") for i in range(OC)]
        kt_sb = [pp.tile([128, S], MF, name=f"kt{i}", tag=f"kt{i}") for i in range(OC)]
        va_sb = [pp.tile([128, HC * 65], MF, name=f"va{i}", tag=f"va{i}") for i in range(TC)]

        # ---------------- Phase A: projections ----------------
        with (
            tc.tile_pool(name="wpool", bufs=10) as wp,
            tc.tile_pool(name="stage", bufs=12) as sp_,
            tc.tile_pool(name="biasp", bufs=1) as bp,
            tc.tile_pool(name="apsum", bufs=4, space="PSUM") as ap_,
        ):
            bq_sb = bp.tile([128, OC], F32, name="bq", tag="bq")
            bk_sb = bp.tile([128, OC], F32, name="bk", tag="bk")
            nc.sync.dma_start(bq_sb[:], bq.rearrange("(c p) -> p c", p=128))
            nc.sync.dma_start(bk_sb[:], bk.rearrange("(c p) -> p c", p=128))
            for i in range(TC):
                nc.gpsimd.memset(
                    va_sb[i].rearrange("p (h w) -> p h w", w=65)[:, :, 64:65]
                    .bitcast(mybir.dt.uint16),
                    0x3F80,
                )

            for name, srcd, wd, dst, bias in (
                ("wq", qT, wq, qt_sb, bq_sb),
                ("wk", kT, wk, kt_sb, bk_sb),
                ("wv", vT, wv, None, None),
            ):
                wch = []
                for i in range(8):
                    w = wp.tile([128, DHC], MF, name="w", tag="w")
                    nc.sync.dma_start(w[:], wd[i * 128:(i + 1) * 128, :])
                    wch.append(w)
                for tb in range(TB):
                    t0 = tb * 512
                    stg = []
                    for Dc in range(8):
                        st = sp_.tile([128, 512], MF, name="stage", tag="stage")
                        nc.sync.dma_start(
                            st[:], srcd[Dc * 128:(Dc + 1) * 128, t0:t0 + 512]
                        )
                        stg.append(st)
                    if name != "wv":
                        for oc in range(OC):
                            ps = ap_.tile([128, 512], F32, name="aps", tag="aps")
                            for Dc in range(8):
                                nc.tensor.matmul(
                                    ps[:],
                                    r(wch[Dc][:, oc * 128:(oc + 1) * 128]),
                                    r(stg[Dc][:]),
                                    start=(Dc == 0),
                                    stop=(Dc == 7),
                                )
                            nc.vector.tensor_scalar_add(
                                dst[oc][:, t0:t0 + 512], ps[:], bias[:, oc:oc + 1]
                            )
                    else:
                        for ti in range(4):
                            tcn = tb * 4 + ti
                            ps = ap_.tile([128, DHC], F32, name="aps", tag="aps")
                            for Dc in range(8):
                                nc.tensor.matmul(
                                    ps[:],
                                    r(stg[Dc][:, ti * 128:(ti + 1) * 128]),
                                    r(wch[Dc][:]),
                                    start=(Dc == 0),
                                    stop=(Dc == 7),
                                )
                            nc.scalar.copy(
                                va_sb[tcn].rearrange("p (h w) -> p h w", w=65)[:, :, 0:64],
                                ps.rearrange("p (h w) -> p h w", w=64)[:, :, :],
                            )

        # ---------------- Phase B: attention ----------------
        with (
            tc.tile_pool(name="maskp", bufs=3) as mp_,
            tc.tile_pool(name="ptp", bufs=3) as ptp,
            tc.tile_pool(name="ostage", bufs=2) as osp,
            tc.tile_pool(name="spsum", bufs=2, space="PSUM") as sp2,
            tc.tile_pool(name="opsum", bufs=1, space="PSUM") as op2,
        ):
            for h in range(HC):
                hc, hp = h // 2, (h % 2) * 64
                ots = [op2.tile([65, 1024], F32, name=f"ot{qb}", tag=f"ot{qb}")
                       for qb in range(QB)]
                for kc in range(KC):
                    mt = mp_.tile([128, S], U8, name="mask", tag="mask")
                    nc.sync.dma_start(mt[:], mk[h, kc])
                    for qb in range(QB):
                        q0 = qb * 1024
                        st = sp2.tile([128, 1024], F32, name="st", tag="st")
                        for j in range(2):
                            nc.tensor.matmul(
                                st[:, j * 512:(j + 1) * 512],
                                r(kt_sb[hc][hp:hp + 64, kc * 128:(kc + 1) * 128]),
                                r(qt_sb[hc][hp:hp + 64, q0 + j * 512:q0 + (j + 1) * 512]),
                                start=True,
                                stop=True,
                            )
                        pt = ptp.tile([128, 1024], MF, name="pt", tag="pt")
                        nc.scalar.activation(pt[:], st[:], Exp, scale=0.125)
                        nc.vector.tensor_tensor(
                            pt[:], pt[:], mt[:, q0:q0 + 1024], op=mult
                        )
                        for j in range(2):
                            nc.tensor.matmul(
                                ots[qb][:, j * 512:(j + 1) * 512],
                                r(va_sb[kc][:, h * 65:h * 65 + 65]),
                                r(pt[:, j * 512:(j + 1) * 512]),
                                start=(kc == 0),
                                stop=(kc == KC - 1),
                                skip_group_check=True,
                            )
                for qb in range(QB):
                    q0 = qb * 1024
                    stg = osp.tile([65, 1024], OF, name="ostage", tag="ostage")
                    nc.scalar.copy(stg[:], ots[qb][:])
                    nc.sync.dma_start(
                        otn_sb[hc][hp:hp + 64, q0:q0 + 1024], stg[0:64, :]
                    )
                    nc.sync.dma_start(
                        sums_sb[h:h + 1, q0:q0 + 1024], stg[64:65, :]
                    )

        qkv_scope.__exit__(None, None, None)

        # ---------------- Phase C: normalize + output projection ----------------
        with (
            tc.tile_pool(name="rcp", bufs=1) as rcp,
            tc.tile_pool(name="wop", bufs=1) as wop,
            tc.tile_pool(name="ystage", bufs=3) as ysp,
            tc.tile_pool(name="cpsum", bufs=4, space="PSUM") as cp_,
        ):
            recip_sb = rcp.tile([HC, S], OF, name="recip", tag="recip")
            with nc.allow_low_precision(reason="fp32r holds full fp32 reciprocal"):
                nc.vector.reciprocal(recip_sb[:], sums_sb[:])
            pat_sb = rcp.tile([HC, DHC], OF, name="pat", tag="pat")
            nc.sync.dma_start(pat_sb[:], patd[:])
            pats = [pat_sb[:, dc * 128:(dc + 1) * 128] for dc in range(OC)]
            for dc in range(OC):
                for qs in range(4):
                    rps = cp_.tile([128, 512], F32, name="cps", tag="cps")
                    nc.tensor.matmul(
                        rps[:], r(pats[dc][:]), r(recip_sb[:, qs * 512:(qs + 1) * 512]),
                        start=True, stop=True,
                    )
                    nc.vector.tensor_tensor(
                        otn_sb[dc][:, qs * 512:(qs + 1) * 512],
                        otn_sb[dc][:, qs * 512:(qs + 1) * 512],
                        rps[:],
                        op=mult,
                    )
            wo_sb = [wop.tile([128, D], OF, name=f"wo{i}", tag=f"wo{i}") for i in range(OC)]
            for i in range(OC):
                nc.sync.dma_start(wo_sb[i][:], wo[i * 128:(i + 1) * 128, :])
            for t in range(TC):
                ys = ysp.tile([128, D], F32, name="ys", tag="ys")
                for half in range(2):
                    yps = cp_.tile([128, 512], F32, name="cps", tag="cps")
                    for dc in range(OC):
                        nc.tensor.matmul(
                            yps[:],
                            r(otn_sb[dc][:, t * 128:(t + 1) * 128]),
                            r(wo_sb[dc][:, half * 512:(half + 1) * 512]),
                            start=(dc == 0),
                            stop=(dc == OC - 1),
                        )
                    nc.scalar.copy(ys[:, half * 512:(half + 1) * 512], yps[:])
                nc.sync.dma_start(y[t * 128:(t + 1) * 128, :], ys[:])


def _build():
    if "nc" in _CACHE:
        return _CACHE["nc"]
    import concourse.mybir as mybir
    import concourse.tile as tile
    from concourse import bacc

    nc = bacc.Bacc(None, target_bir_lowering=False)
    with tile.TileContext(nc) as tc:
        _emit(nc, tc, tile, mybir)
    nc.compile()
    _CACHE["nc"] = nc
    return nc


def _pattern():
    """[h, d] -> 1.0 where global row dc*128+(d%128) belongs to head h."""
    p = np.zeros((HC, DHC), dtype=np.float32)
    for d in range(DHC):
        p[d // DH, d] = 1.0
    return p


def _prep_inputs(query, key, value, mask, Wq, Wk, Wv, Wo, bq, bk):
    """Build the 8 per-core input maps (host-side sharding/relayout)."""
    import ml_dtypes
    bf16 = ml_dtypes.bfloat16
    in_maps = []
    qT = [np.ascontiguousarray(query[b].T).astype(bf16) for b in range(B)]
    kTt = [np.ascontiguousarray(key[b].T).astype(bf16) for b in range(B)]
    vT = [np.ascontiguousarray(value[b].T).astype(bf16) for b in range(B)]
    # keep-mask, transposed to [b][h, k, q] then chunked [h, kc, 128, q]
    keep = (~mask).astype(np.uint8).reshape(B, S, H, S)
    for c in range(NCORES):
        b, g = c // 2, c % 2
        hs = slice(g * HC * DH, (g + 1) * HC * DH)
        mkc = np.ascontiguousarray(
            keep[b, :, g * HC:(g + 1) * HC, :].transpose(1, 2, 0)
        ).reshape(HC, KC, 128, S)
        in_maps.append({
            "qT": qT[b], "kT": kTt[b], "vT": vT[b], "mk": mkc,
            "wq": np.ascontiguousarray(Wq[:, hs]).astype(bf16),
            "wk": np.ascontiguousarray(Wk[:, hs]).astype(bf16),
            "wv": np.ascontiguousarray(Wv[:, hs]).astype(bf16),
            "wo": np.ascontiguousarray(Wo[hs, :]),
            "bq": np.ascontiguousarray(bq[hs]),
            "bk": np.ascontiguousarray(bk[hs]),
            "patd": _pattern(),
        })
    return in_maps


def kernel(query, key, value, mask, Wq, bq, Wk, bk, Wv, bv, Wo, bo, _trace=False):
    from concourse.bass_utils import run_bass_kernel_spmd

    query = np.asarray(query, dtype=np.float32)
    key = np.asarray(key, dtype=np.float32)
    value = np.asarray(value, dtype=np.float32)
    mask = np.asarray(mask)
    Wq = np.asarray(Wq, dtype=np.float32)
    Wk = np.asarray(Wk, dtype=np.float32)
    Wv = np.asarray(Wv, dtype=np.float32)
    Wo = np.asarray(Wo, dtype=np.float32)
    bq = np.asarray(bq, dtype=np.float32)
    bk = np.asarray(bk, dtype=np.float32)
    bv = np.asarray(bv, dtype=np.float32)
    bo = np.asarray(bo, dtype=np.float32)

    nc = _build()
    in_maps = _prep_inputs(query, key, value, mask, Wq, Wk, Wv, Wo, bq, bk)
    kwargs = {}
    if _trace:
        kwargs = {"trace": True}
    res = run_bass_kernel_spmd(nc, in_maps, core_ids=list(range(NCORES)), **kwargs)
    # attention weights sum to 1, so the V bias contributes bv @ Wo verbatim
    const_vec = (bv @ Wo + bo).astype(np.float32)
    out = np.empty((B, S, D), dtype=np.float32)
    for b in range(B):
        out[b] = res.results[2 * b]["y"] + res.results[2 * b + 1]["y"] + const_vec
    if _trace:
        return out, res
    return out


# revision 13
# speedup vs baseline: 1.8742x; 1.0818x over previous
"""Multi-head attention Bass kernel for Trainium2, 8-core SPMD.

Problem: B=4, S=2048, D=1024, H=16 heads (dh=64), boolean attention mask.
  out = softmax(mask ? -1e9 : (Q Kt / 8)) V -> @ Wo + bo

Sharding: 8 cores = 4 batches x 2 head-groups. Core c handles batch c//2,
heads [8*(c%2), 8*(c%2)+8). Host transposes activations to [D, S] so the
PE (which contracts along partitions) consumes them directly; the keep-mask
(~mask) is host-transposed to [head, kchunk, k, q] uint8. Per-core partial
outputs Y_c = (attn_heads @ Wo_slice) are summed pairwise on the host
(tensor-parallel "all-reduce"), and bias terms (bv@Wo + bo) are added once.

Device pipeline per core (all matmuls in fp32r: full PE rate, fp32 storage):
  A) QT/KT = Wq/Wk slices applied to queryT/keyT (+bias via DVE);
     V[t,d] = valueT chunks (stationary) x Wv (moving), stored with a
     ones-column per head (65-stride) so PV emits row-sums for free.
  B) per (head, q-block, k-chunk): S^T = KT' QT -> PSUM; ACT exp(s/8);
     DVE multiply by keep-mask (u8); PV accumulates O^T[65, q] where
     row 64 = sum_k exp. Evict via SBUF stage + DMA (partition relayout).
  C) recip = 1/sums; broadcast per-head recip rows onto 128 partitions with
     a 0/1 pattern matmul; normalize O^T in place; Y = O^T chunks @ Wo.
"""

import sys

if "/opt/trn_rl_repo" not in sys.path:
    sys.path.insert(0, "/opt/trn_rl_repo")

import numpy as np

B, S, D, H = 4, 2048, 1024, 16
DH = D // H            # 64 per-head dim
HC = H // 2            # 8 heads per core
DHC = HC * DH          # 512 per-core head dims
NCORES = 8
KC = S // 128          # 16 k chunks
OC = DHC // 128        # 4 output chunks for Q/K projections
TC = S // 128          # 16 t chunks
TB = S // 512          # 4 t blocks for projection staging
QB = S // 1024         # 2 q blocks in attention

_CACHE = {}


def _emit(nc, tc, tile, mybir):
    F32 = mybir.dt.float32
    U8 = mybir.dt.uint8
    F32R = mybir.dt.float32r
    mult = mybir.AluOpType.mult
    Exp = mybir.ActivationFunctionType.Exp

    BF = mybir.dt.bfloat16
    MF = BF          # attention-side matmul operand dtype
    OF = F32R        # output-path matmul operand dtype

    def r(ap):
        return ap

    qT = nc.dram_tensor("qT", [D, S], MF, kind="ExternalInput")
    kT = nc.dram_tensor("kT", [D, S], MF, kind="ExternalInput")
    vT = nc.dram_tensor("vT", [D, S], MF, kind="ExternalInput")
    mk = nc.dram_tensor("mk", [HC, KC, 128, S], U8, kind="ExternalInput")
    wq = nc.dram_tensor("wq", [D, DHC], MF, kind="ExternalInput")
    wk = nc.dram_tensor("wk", [D, DHC], MF, kind="ExternalInput")
    wv = nc.dram_tensor("wv", [D, DHC], MF, kind="ExternalInput")
    wo = nc.dram_tensor("wo", [DHC, D], OF, kind="ExternalInput")
    bq = nc.dram_tensor("bq", [DHC], F32, kind="ExternalInput")
    bk = nc.dram_tensor("bk", [DHC], F32, kind="ExternalInput")
    patd = nc.dram_tensor("patd", [HC, DHC], OF, kind="ExternalInput")
    y = nc.dram_tensor("y", [S, D], F32, kind="ExternalOutput")

    with tc.tile_pool(name="longp", bufs=1) as lp:
        sums_sb = lp.tile([HC, S], OF, name="sums", tag="sums")
        otn_sb = [lp.tile([128, S], OF, name=f"otn{i}", tag=f"otn{i}") for i in range(OC)]
        qkv_scope = tc.tile_pool(name="qkvp", bufs=1)
        pp = qkv_scope.__enter__()
        qt_sb = [pp.tile([128, S], MF, name=f"qt{i}", tag=f"qt{i}") for i in range(OC)]
        kt_sb = [pp.tile([128, S], MF, name=f"kt{i}", tag=f"kt{i}") for i in range(OC)]
        va_sb = [pp.tile([128, HC * 65], MF, name=f"va{i}", tag=f"va{i}") for i in range(TC)]

        # ---------------- Phase A: projections ----------------
        with (
            tc.tile_pool(name="wpool", bufs=10) as wp,
            tc.tile_pool(name="stage", bufs=12) as sp_,
            tc.tile_pool(name="biasp", bufs=1) as bp,
            tc.tile_pool(name="apsum", bufs=4, space="PSUM") as ap_,
        ):
            bq_sb = bp.tile([128, OC], F32, name="bq", tag="bq")
            bk_sb = bp.tile([128, OC], F32, name="bk", tag="bk")
            nc.sync.dma_start(bq_sb[:], bq.rearrange("(c p) -> p c", p=128))
            nc.sync.dma_start(bk_sb[:], bk.rearrange("(c p) -> p c", p=128))
            for i in range(TC):
                nc.gpsimd.memset(
                    va_sb[i].rearrange("p (h w) -> p h w", w=65)[:, :, 64:65]
                    .bitcast(mybir.dt.uint16),
                    0x3F80,
                )

            for name, srcd, wd, dst, bias in (
                ("wq", qT, wq, qt_sb, bq_sb),
                ("wk", kT, wk, kt_sb, bk_sb),
                ("wv", vT, wv, None, None),
            ):
                wch = []
                for i in range(8):
                    w = wp.tile([128, DHC], MF, name="w", tag="w")
                    nc.sync.dma_start(w[:], wd[i * 128:(i + 1) * 128, :])
                    wch.append(w)
                for tb in range(TB):
                    t0 = tb * 512
                    stg = []
                    for Dc in range(8):
                        st = sp_.tile([128, 512], MF, name="stage", tag="stage")
                        nc.sync.dma_start(
                            st[:], srcd[Dc * 128:(Dc + 1) * 128, t0:t0 + 512]
                        )
                        stg.append(st)
                    if name != "wv":
                        for oc in range(OC):
                            ps = ap_.tile([128, 512], F32, name="aps", tag="aps")
                            for Dc in range(8):
                                nc.tensor.matmul(
                                    ps[:],
                                    r(wch[Dc][:, oc * 128:(oc + 1) * 128]),
                                    r(stg[Dc][:]),
                                    start=(Dc == 0),
                                    stop=(Dc == 7),
                                )
                            nc.vector.tensor_scalar_add(
                                dst[oc][:, t0:t0 + 512], ps[:], bias[:, oc:oc + 1]
                            )
                    else:
                        for ti in range(4):
                            tcn = tb * 4 + ti
                            ps = ap_.tile([128, DHC], F32, name="aps", tag="aps")
                            for Dc in range(8):
                                nc.tensor.matmul(
                                    ps[:],
                                    r(stg[Dc][:, ti * 128:(ti + 1) * 128]),
                                    r(wch[Dc][:]),
                                    start=(Dc == 0),
                                    stop=(Dc == 7),
                                )
                            nc.scalar.copy(
                                va_sb[tcn].rearrange("p (h w) -> p h w", w=65)[:, :, 0:64],
                                ps.rearrange("p (h w) -> p h w", w=64)[:, :, :],
                            )

        # ---------------- Phase B: attention ----------------
        with (
            tc.tile_pool(name="maskp", bufs=3) as mp_,
            tc.tile_pool(name="ptp", bufs=5) as ptp,
            tc.tile_pool(name="ostage", bufs=2) as osp,
            tc.tile_pool(name="spsum", bufs=2, space="PSUM") as sp2,
            tc.tile_pool(name="opsum", bufs=1, space="PSUM") as op2,
        ):
            for h in range(HC):
                hc, hp = h // 2, (h % 2) * 64
                ots = [op2.tile([65, 1024], F32, name=f"ot{qb}", tag=f"ot{qb}")
                       for qb in range(QB)]
                def emit_pv(kc, pts):
                    for qb in range(QB):
                        for j in range(2):
                            nc.tensor.matmul(
                                ots[qb][:, j * 512:(j + 1) * 512],
                                r(va_sb[kc][:, h * 65:h * 65 + 65]),
                                r(pts[qb][:, j * 512:(j + 1) * 512]),
                                start=(kc == 0),
                                stop=(kc == KC - 1),
                                skip_group_check=True,
                            )

                prev = None
                for kc in range(KC):
                    mt = mp_.tile([128, S], U8, name="mask", tag="mask")
                    nc.sync.dma_start(mt[:], mk[h, kc])
                    cur = []
                    for qb in range(QB):
                        q0 = qb * 1024
                        st = sp2.tile([128, 1024], F32, name="st", tag="st")
                        for j in range(2):
                            nc.tensor.matmul(
                                st[:, j * 512:(j + 1) * 512],
                                r(kt_sb[hc][hp:hp + 64, kc * 128:(kc + 1) * 128]),
                                r(qt_sb[hc][hp:hp + 64, q0 + j * 512:q0 + (j + 1) * 512]),
                                start=True,
                                stop=True,
                            )
                        pt = ptp.tile([128, 1024], MF, name="pt", tag="pt")
                        nc.scalar.activation(pt[:], st[:], Exp, scale=0.125)
                        nc.vector.tensor_tensor(
                            pt[:], pt[:], mt[:, q0:q0 + 1024], op=mult
                        )
                        cur.append(pt)
                    if prev is not None:
                        emit_pv(kc - 1, prev)
                    prev = cur
                emit_pv(KC - 1, prev)
                for qb in range(QB):
                    q0 = qb * 1024
                    stg = osp.tile([65, 1024], OF, name="ostage", tag="ostage")
                    nc.scalar.copy(stg[:], ots[qb][:])
                    nc.sync.dma_start(
                        otn_sb[hc][hp:hp + 64, q0:q0 + 1024], stg[0:64, :]
                    )
                    nc.sync.dma_start(
                        sums_sb[h:h + 1, q0:q0 + 1024], stg[64:65, :]
                    )

        qkv_scope.__exit__(None, None, None)

        # ---------------- Phase C: normalize + output projection ----------------
        with (
            tc.tile_pool(name="rcp", bufs=1) as rcp,
            tc.tile_pool(name="wop", bufs=1) as wop,
            tc.tile_pool(name="ystage", bufs=3) as ysp,
            tc.tile_pool(name="cpsum", bufs=4, space="PSUM") as cp_,
        ):
            recip_sb = rcp.tile([HC, S], OF, name="recip", tag="recip")
            with nc.allow_low_precision(reason="fp32r holds full fp32 reciprocal"):
                nc.vector.reciprocal(recip_sb[:], sums_sb[:])
            pat_sb = rcp.tile([HC, DHC], OF, name="pat", tag="pat")
            nc.sync.dma_start(pat_sb[:], patd[:])
            pats = [pat_sb[:, dc * 128:(dc + 1) * 128] for dc in range(OC)]
            for dc in range(OC):
                for qs in range(4):
                    rps = cp_.tile([128, 512], F32, name="cps", tag="cps")
                    nc.tensor.matmul(
                        rps[:], r(pats[dc][:]), r(recip_sb[:, qs * 512:(qs + 1) * 512]),
                        start=True, stop=True,
                    )
                    nc.vector.tensor_tensor(
                        otn_sb[dc][:, qs * 512:(qs + 1) * 512],
                        otn_sb[dc][:, qs * 512:(qs + 1) * 512],
                        rps[:],
                        op=mult,
                    )
            wo_sb = [wop.tile([128, D], OF, name=f"wo{i}", tag=f"wo{i}") for i in range(OC)]
            for i in range(OC):
                nc.sync.dma_start(wo_sb[i][:], wo[i * 128:(i + 1) * 128, :])
            for t in range(TC):
                ys = ysp.tile([128, D], F32, name="ys", tag="ys")
                for half in range(2):
                    yps = cp_.tile([128, 512], F32, name="cps", tag="cps")
                    for dc in range(OC):
                        nc.tensor.matmul(
                            yps[:],
                            r(otn_sb[dc][:, t * 128:(t + 1) * 128]),
                            r(wo_sb[dc][:, half * 512:(half + 1) * 512]),
                            start=(dc == 0),
                            stop=(dc == OC - 1),
                        )
                    nc.scalar.copy(ys[:, half * 512:(half + 1) * 512], yps[:])
                nc.sync.dma_start(y[t * 128:(t + 1) * 128, :], ys[:])


def _build():
    if "nc" in _CACHE:
        return _CACHE["nc"]
    import concourse.mybir as mybir
    import concourse.tile as tile
    from concourse import bacc

    nc = bacc.Bacc(None, target_bir_lowering=False)
    with tile.TileContext(nc) as tc:
        _emit(nc, tc, tile, mybir)
    nc.compile()
    _CACHE["nc"] = nc
    return nc


def _pattern():
    """[h, d] -> 1.0 where global row dc*128+(d%128) belongs to head h."""
    p = np.zeros((HC, DHC), dtype=np.float32)
    for d in range(DHC):
        p[d // DH, d] = 1.0
    return p


def _prep_inputs(query, key, value, mask, Wq, Wk, Wv, Wo, bq, bk):
    """Build the 8 per-core input maps (host-side sharding/relayout)."""
    import ml_dtypes
    bf16 = ml_dtypes.bfloat16
    in_maps = []
    qT = [np.ascontiguousarray(query[b].T).astype(bf16) for b in range(B)]
    kTt = [np.ascontiguousarray(key[b].T).astype(bf16) for b in range(B)]
    vT = [np.ascontiguousarray(value[b].T).astype(bf16) for b in range(B)]
    # keep-mask, transposed to [b][h, k, q] then chunked [h, kc, 128, q]
    keep = (~mask).astype(np.uint8).reshape(B, S, H, S)
    for c in range(NCORES):
        b, g = c // 2, c % 2
        hs = slice(g * HC * DH, (g + 1) * HC * DH)
        mkc = np.ascontiguousarray(
            keep[b, :, g * HC:(g + 1) * HC, :].transpose(1, 2, 0)
        ).reshape(HC, KC, 128, S)
        in_maps.append({
            "qT": qT[b], "kT": kTt[b], "vT": vT[b], "mk": mkc,
            "wq": np.ascontiguousarray(Wq[:, hs]).astype(bf16),
            "wk": np.ascontiguousarray(Wk[:, hs]).astype(bf16),
            "wv": np.ascontiguousarray(Wv[:, hs]).astype(bf16),
            "wo": np.ascontiguousarray(Wo[hs, :]),
            "bq": np.ascontiguousarray(bq[hs]),
            "bk": np.ascontiguousarray(bk[hs]),
            "patd": _pattern(),
        })
    return in_maps


def kernel(query, key, value, mask, Wq, bq, Wk, bk, Wv, bv, Wo, bo, _trace=False):
    from concourse.bass_utils import run_bass_kernel_spmd

    query = np.asarray(query, dtype=np.float32)
    key = np.asarray(key, dtype=np.float32)
    value = np.asarray(value, dtype=np.float32)
    mask = np.asarray(mask)
    Wq = np.asarray(Wq, dtype=np.float32)
    Wk = np.asarray(Wk, dtype=np.float32)
    Wv = np.asarray(Wv, dtype=np.float32)
    Wo = np.asarray(Wo, dtype=np.float32)
    bq = np.asarray(bq, dtype=np.float32)
    bk = np.asarray(bk, dtype=np.float32)
    bv = np.asarray(bv, dtype=np.float32)
    bo = np.asarray(bo, dtype=np.float32)

    nc = _build()
    in_maps = _prep_inputs(query, key, value, mask, Wq, Wk, Wv, Wo, bq, bk)
    kwargs = {}
    if _trace:
        kwargs = {"trace": True}
    res = run_bass_kernel_spmd(nc, in_maps, core_ids=list(range(NCORES)), **kwargs)
    # attention weights sum to 1, so the V bias contributes bv @ Wo verbatim
    const_vec = (bv @ Wo + bo).astype(np.float32)
    out = np.empty((B, S, D), dtype=np.float32)
    for b in range(B):
        out[b] = res.results[2 * b]["y"] + res.results[2 * b + 1]["y"] + const_vec
    if _trace:
        return out, res
    return out


# revision 15
# speedup vs baseline: 1.9483x; 1.0395x over previous
"""Multi-head attention Bass kernel for Trainium2, 8-core SPMD.

Problem: B=4, S=2048, D=1024, H=16 heads (dh=64), boolean attention mask.
  out = softmax(mask ? -1e9 : (Q Kt / 8)) V -> @ Wo + bo

Sharding: 8 cores = 4 batches x 2 head-groups. Core c handles batch c//2,
heads [8*(c%2), 8*(c%2)+8). Host transposes activations to [D, S] so the
PE (which contracts along partitions) consumes them directly; the keep-mask
(~mask) is host-transposed to [head, kchunk, k, q] uint8. Per-core partial
outputs Y_c = (attn_heads @ Wo_slice) are summed pairwise on the host
(tensor-parallel "all-reduce"), and bias terms (bv@Wo + bo) are added once.

Device pipeline per core (all matmuls in fp32r: full PE rate, fp32 storage):
  A) QT/KT = Wq/Wk slices applied to queryT/keyT (+bias via DVE);
     V[t,d] = valueT chunks (stationary) x Wv (moving), stored with a
     ones-column per head (65-stride) so PV emits row-sums for free.
  B) per (head, q-block, k-chunk): S^T = KT' QT -> PSUM; ACT exp(s/8);
     DVE multiply by keep-mask (u8); PV accumulates O^T[65, q] where
     row 64 = sum_k exp. Evict via SBUF stage + DMA (partition relayout).
  C) recip = 1/sums; broadcast per-head recip rows onto 128 partitions with
     a 0/1 pattern matmul; normalize O^T in place; Y = O^T chunks @ Wo.
"""

import sys

if "/opt/trn_rl_repo" not in sys.path:
    sys.path.insert(0, "/opt/trn_rl_repo")

import numpy as np

B, S, D, H = 4, 2048, 1024, 16
DH = D // H            # 64 per-head dim
HC = H // 2            # 8 heads per core
DHC = HC * DH          # 512 per-core head dims
NCORES = 8
KC = S // 128          # 16 k chunks
OC = DHC // 128        # 4 output chunks for Q/K projections
TC = S // 128          # 16 t chunks
TB = S // 512          # 4 t blocks for projection staging
QB = S // 1024         # 2 q blocks in attention

_CACHE = {}


def _emit(nc, tc, tile, mybir):
    F32 = mybir.dt.float32
    U8 = mybir.dt.uint8
    F32R = mybir.dt.float32r
    mult = mybir.AluOpType.mult
    Exp = mybir.ActivationFunctionType.Exp

    BF = mybir.dt.bfloat16
    MF = BF          # attention-side matmul operand dtype
    OF = F32R        # output-path matmul operand dtype

    def r(ap):
        return ap

    qT = nc.dram_tensor("qT", [D, S], MF, kind="ExternalInput")
    kT = nc.dram_tensor("kT", [D, S], MF, kind="ExternalInput")
    vT = nc.dram_tensor("vT", [D, S], MF, kind="ExternalInput")
    F8 = mybir.dt.float8e4
    mk = nc.dram_tensor("mk", [HC, KC, 128, S], F8, kind="ExternalInput")
    identd = nc.dram_tensor("identd", [128, 128], F8, kind="ExternalInput")
    wq = nc.dram_tensor("wq", [D, DHC], MF, kind="ExternalInput")
    wk = nc.dram_tensor("wk", [D, DHC], MF, kind="ExternalInput")
    wv = nc.dram_tensor("wv", [D, DHC], MF, kind="ExternalInput")
    wo = nc.dram_tensor("wo", [DHC, D], OF, kind="ExternalInput")
    bq = nc.dram_tensor("bq", [DHC], F32, kind="ExternalInput")
    bk = nc.dram_tensor("bk", [DHC], F32, kind="ExternalInput")
    patd = nc.dram_tensor("patd", [HC, DHC], OF, kind="ExternalInput")
    y = nc.dram_tensor("y", [S, D], F32, kind="ExternalOutput")

    with tc.tile_pool(name="longp", bufs=1) as lp:
        sums_sb = lp.tile([HC, S], OF, name="sums", tag="sums")
        otn_sb = [lp.tile([128, S], OF, name=f"otn{i}", tag=f"otn{i}") for i in range(OC)]
        qkv_scope = tc.tile_pool(name="qkvp", bufs=1)
        pp = qkv_scope.__enter__()
        qt_sb = [pp.tile([128, S], MF, name=f"qt{i}", tag=f"qt{i}") for i in range(OC)]
        kt_sb = [pp.tile([128, S], MF, name=f"kt{i}", tag=f"kt{i}") for i in range(OC)]
        va_sb = [pp.tile([128, HC * 65], MF, name=f"va{i}", tag=f"va{i}") for i in range(TC)]

        # ---------------- Phase A: projections ----------------
        with (
            tc.tile_pool(name="wpool", bufs=10) as wp,
            tc.tile_pool(name="stage", bufs=12) as sp_,
            tc.tile_pool(name="biasp", bufs=1) as bp,
            tc.tile_pool(name="apsum", bufs=4, space="PSUM") as ap_,
        ):
            bq_sb = bp.tile([128, OC], F32, name="bq", tag="bq")
            bk_sb = bp.tile([128, OC], F32, name="bk", tag="bk")
            nc.sync.dma_start(bq_sb[:], bq.rearrange("(c p) -> p c", p=128))
            nc.sync.dma_start(bk_sb[:], bk.rearrange("(c p) -> p c", p=128))
            for i in range(TC):
                nc.gpsimd.memset(
                    va_sb[i].rearrange("p (h w) -> p h w", w=65)[:, :, 64:65]
                    .bitcast(mybir.dt.uint16),
                    0x3F80,
                )

            for name, srcd, wd, dst, bias in (
                ("wq", qT, wq, qt_sb, bq_sb),
                ("wk", kT, wk, kt_sb, bk_sb),
                ("wv", vT, wv, None, None),
            ):
                wch = []
                for i in range(8):
                    w = wp.tile([128, DHC], MF, name="w", tag="w")
                    nc.sync.dma_start(w[:], wd[i * 128:(i + 1) * 128, :])
                    wch.append(w)
                for tb in range(TB):
                    t0 = tb * 512
                    stg = []
                    for Dc in range(8):
                        st = sp_.tile([128, 512], MF, name="stage", tag="stage")
                        nc.sync.dma_start(
                            st[:], srcd[Dc * 128:(Dc + 1) * 128, t0:t0 + 512]
                        )
                        stg.append(st)
                    if name != "wv":
                        for oc in range(OC):
                            ps = ap_.tile([128, 512], F32, name="aps", tag="aps")
                            for Dc in range(8):
                                nc.tensor.matmul(
                                    ps[:],
                                    r(wch[Dc][:, oc * 128:(oc + 1) * 128]),
                                    r(stg[Dc][:]),
                                    start=(Dc == 0),
                                    stop=(Dc == 7),
                                )
                            nc.vector.tensor_scalar_add(
                                dst[oc][:, t0:t0 + 512], ps[:], bias[:, oc:oc + 1]
                            )
                    else:
                        for ti in range(4):
                            tcn = tb * 4 + ti
                            ps = ap_.tile([128, DHC], F32, name="aps", tag="aps")
                            for Dc in range(8):
                                nc.tensor.matmul(
                                    ps[:],
                                    r(stg[Dc][:, ti * 128:(ti + 1) * 128]),
                                    r(wch[Dc][:]),
                                    start=(Dc == 0),
                                    stop=(Dc == 7),
                                )
                            nc.scalar.copy(
                                va_sb[tcn].rearrange("p (h w) -> p h w", w=65)[:, :, 0:64],
                                ps.rearrange("p (h w) -> p h w", w=64)[:, :, :],
                            )

        # ---------------- Phase B: attention ----------------
        with (
            tc.tile_pool(name="maskp", bufs=3) as mp_,
            tc.tile_pool(name="identp", bufs=1) as idp,
            tc.tile_pool(name="ptp", bufs=5) as ptp,
            tc.tile_pool(name="ostage", bufs=2) as osp,
            tc.tile_pool(name="spsum", bufs=2, space="PSUM") as sp2,
            tc.tile_pool(name="opsum", bufs=1, space="PSUM") as op2,
        ):
            ident_sb = idp.tile([128, 128], F8, name="ident", tag="ident")
            nc.sync.dma_start(ident_sb[:], identd[:])
            for h in range(HC):
                hc, hp = h // 2, (h % 2) * 64
                ots = [op2.tile([65, 1024], F32, name=f"ot{qb}", tag=f"ot{qb}")
                       for qb in range(QB)]
                def emit_pv(kc, pts):
                    for qb in range(QB):
                        for j in range(2):
                            nc.tensor.matmul(
                                ots[qb][:, j * 512:(j + 1) * 512],
                                r(va_sb[kc][:, h * 65:h * 65 + 65]),
                                r(pts[qb][:, j * 512:(j + 1) * 512]),
                                start=(kc == 0),
                                stop=(kc == KC - 1),
                                skip_group_check=True,
                            )

                prev = None
                for kc in range(KC):
                    mt = mp_.tile([128, S], F8, name="mask", tag="mask")
                    nc.sync.dma_start(mt[:], mk[h, kc])
                    cur = []
                    for qb in range(QB):
                        q0 = qb * 1024
                        st = sp2.tile([128, 1024], F32, name="st", tag="st")
                        for j in range(2):
                            nc.tensor.matmul(
                                st[:, j * 512:(j + 1) * 512],
                                r(kt_sb[hc][hp:hp + 64, kc * 128:(kc + 1) * 128]),
                                r(qt_sb[hc][hp:hp + 64, q0 + j * 512:q0 + (j + 1) * 512]),
                                start=True,
                                stop=False,
                                skip_group_check=True,
                            )
                        for j in range(2):
                            nc.tensor.matmul(
                                st[:, j * 512:(j + 1) * 512],
                                ident_sb[:],
                                mt[:, q0 + j * 512:q0 + (j + 1) * 512],
                                start=False,
                                stop=True,
                                skip_group_check=True,
                            )
                        pt = ptp.tile([128, 1024], MF, name="pt", tag="pt")
                        nc.scalar.activation(pt[:], st[:], Exp, scale=0.125)
                        cur.append(pt)
                    if prev is not None:
                        emit_pv(kc - 1, prev)
                    prev = cur
                emit_pv(KC - 1, prev)
                for qb in range(QB):
                    q0 = qb * 1024
                    stg = osp.tile([65, 1024], OF, name="ostage", tag="ostage")
                    nc.scalar.copy(stg[:], ots[qb][:])
                    nc.sync.dma_start(
                        otn_sb[hc][hp:hp + 64, q0:q0 + 1024], stg[0:64, :]
                    )
                    nc.sync.dma_start(
                        sums_sb[h:h + 1, q0:q0 + 1024], stg[64:65, :]
                    )

        qkv_scope.__exit__(None, None, None)

        # ---------------- Phase C: normalize + output projection ----------------
        with (
            tc.tile_pool(name="rcp", bufs=1) as rcp,
            tc.tile_pool(name="wop", bufs=1) as wop,
            tc.tile_pool(name="ystage", bufs=3) as ysp,
            tc.tile_pool(name="cpsum", bufs=4, space="PSUM") as cp_,
        ):
            recip_sb = rcp.tile([HC, S], OF, name="recip", tag="recip")
            with nc.allow_low_precision(reason="fp32r holds full fp32 reciprocal"):
                nc.vector.reciprocal(recip_sb[:], sums_sb[:])
            pat_sb = rcp.tile([HC, DHC], OF, name="pat", tag="pat")
            nc.sync.dma_start(pat_sb[:], patd[:])
            pats = [pat_sb[:, dc * 128:(dc + 1) * 128] for dc in range(OC)]
            for dc in range(OC):
                for qs in range(4):
                    rps = cp_.tile([128, 512], F32, name="cps", tag="cps")
                    nc.tensor.matmul(
                        rps[:], r(pats[dc][:]), r(recip_sb[:, qs * 512:(qs + 1) * 512]),
                        start=True, stop=True,
                    )
                    nc.vector.tensor_tensor(
                        otn_sb[dc][:, qs * 512:(qs + 1) * 512],
                        otn_sb[dc][:, qs * 512:(qs + 1) * 512],
                        rps[:],
                        op=mult,
                    )
            wo_sb = [wop.tile([128, D], OF, name=f"wo{i}", tag=f"wo{i}") for i in range(OC)]
            for i in range(OC):
                nc.sync.dma_start(wo_sb[i][:], wo[i * 128:(i + 1) * 128, :])
            for t in range(TC):
                ys = ysp.tile([128, D], F32, name="ys", tag="ys")
                for half in range(2):
                    yps = cp_.tile([128, 512], F32, name="cps", tag="cps")
                    for dc in range(OC):
                        nc.tensor.matmul(
                            yps[:],
                            r(otn_sb[dc][:, t * 128:(t + 1) * 128]),
                            r(wo_sb[dc][:, half * 512:(half + 1) * 512]),
                            start=(dc == 0),
                            stop=(dc == OC - 1),
                        )
                    nc.scalar.copy(ys[:, half * 512:(half + 1) * 512], yps[:])
                nc.sync.dma_start(y[t * 128:(t + 1) * 128, :], ys[:])


def _build():
    if "nc" in _CACHE:
        return _CACHE["nc"]
    import concourse.mybir as mybir
    import concourse.tile as tile
    from concourse import bacc

    nc = bacc.Bacc(None, target_bir_lowering=False)
    with tile.TileContext(nc) as tc:
        _emit(nc, tc, tile, mybir)
    nc.compile()
    _CACHE["nc"] = nc
    return nc


def _pattern():
    """[h, d] -> 1.0 where global row dc*128+(d%128) belongs to head h."""
    p = np.zeros((HC, DHC), dtype=np.float32)
    for d in range(DHC):
        p[d // DH, d] = 1.0
    return p


def _prep_inputs(query, key, value, mask, Wq, Wk, Wv, Wo, bq, bk):
    """Build the 8 per-core input maps (host-side sharding/relayout)."""
    import ml_dtypes
    bf16 = ml_dtypes.bfloat16
    in_maps = []
    qT = [np.ascontiguousarray(query[b].T).astype(bf16) for b in range(B)]
    kTt = [np.ascontiguousarray(key[b].T).astype(bf16) for b in range(B)]
    vT = [np.ascontiguousarray(value[b].T).astype(bf16) for b in range(B)]
    # excluded-mask (True = masked out), transposed to [h, kc, 128, q], fp8
    f8 = ml_dtypes.float8_e4m3
    keep = mask.astype(np.uint8).reshape(B, S, H, S)
    ident = (np.eye(128, dtype=np.float32) * -240.0).astype(f8)
    for c in range(NCORES):
        b, g = c // 2, c % 2
        hs = slice(g * HC * DH, (g + 1) * HC * DH)
        mkc = np.ascontiguousarray(
            keep[b, :, g * HC:(g + 1) * HC, :].transpose(1, 2, 0)
        ).reshape(HC, KC, 128, S).astype(f8)
        in_maps.append({
            "qT": qT[b], "kT": kTt[b], "vT": vT[b], "mk": mkc,
            "wq": np.ascontiguousarray(Wq[:, hs]).astype(bf16),
            "wk": np.ascontiguousarray(Wk[:, hs]).astype(bf16),
            "wv": np.ascontiguousarray(Wv[:, hs]).astype(bf16),
            "wo": np.ascontiguousarray(Wo[hs, :]),
            "bq": np.ascontiguousarray(bq[hs]),
            "bk": np.ascontiguousarray(bk[hs]),
            "patd": _pattern(),
            "identd": ident,
        })
    return in_maps


def kernel(query, key, value, mask, Wq, bq, Wk, bk, Wv, bv, Wo, bo, _trace=False):
    from concourse.bass_utils import run_bass_kernel_spmd

    query = np.asarray(query, dtype=np.float32)
    key = np.asarray(key, dtype=np.float32)
    value = np.asarray(value, dtype=np.float32)
    mask = np.asarray(mask)
    Wq = np.asarray(Wq, dtype=np.float32)
    Wk = np.asarray(Wk, dtype=np.float32)
    Wv = np.asarray(Wv, dtype=np.float32)
    Wo = np.asarray(Wo, dtype=np.float32)
    bq = np.asarray(bq, dtype=np.float32)
    bk = np.asarray(bk, dtype=np.float32)
    bv = np.asarray(bv, dtype=np.float32)
    bo = np.asarray(bo, dtype=np.float32)

    nc = _build()
    in_maps = _prep_inputs(query, key, value, mask, Wq, Wk, Wv, Wo, bq, bk)
    kwargs = {}
    if _trace:
        kwargs = {"trace": True}
    res = run_bass_kernel_spmd(nc, in_maps, core_ids=list(range(NCORES)), **kwargs)
    # attention weights sum to 1, so the V bias contributes bv @ Wo verbatim
    const_vec = (bv @ Wo + bo).astype(np.float32)
    out = np.empty((B, S, D), dtype=np.float32)
    for b in range(B):
        out[b] = res.results[2 * b]["y"] + res.results[2 * b + 1]["y"] + const_vec
    if _trace:
        return out, res
    return out
